# revision 1
# baseline (speedup 1.0000x reference)
"""ArcFace loss kernel for Trainium2, class-sharded across 8 NeuronCores.

Strategy (vocab/tensor parallel per the module's own sharding):
  - Shard the class axis of `weights` (100000 classes -> 8 x 12800, zero-padded).
  - Each core: normalize its weight shard on device (squares -> ones-matmul
    partition-reduce -> rsqrt via exp(-0.5*ln)), then bf16 matmul
    cos[b, c] = xn . wn with x replicated, then ScalarE exp(64*cos - 64)
    with accum_out producing per-row partial sums of exp.
  - Host: sum the 8 partial exp-sums (f64), fix up the 512 target-class
    entries with the ArcFace margin (cos(theta+m) correction), and take the
    mean cross-entropy.  A fixed shift of -64 (= -S, since cos <= 1) replaces
    the usual running max, so no cross-core max reduction is needed; all
    exp values stay inside normal fp32 range.

Measured (8 NeuronCores, trn2): relative error vs reference 5.9e-05;
per-core HW time ~156us repeat-loop-measured (~148us single-shot after
subtracting the measured 8.5us For_i back-edge), vs an 85us pure-bf16-matmul
roofline for the 6.7 GFLOP/core of logits work.
"""

import math

import ml_dtypes
import numpy as np

# Problem constants (hardcoded per contract; kernel.py must be self-contained).
B = 512  # batch
D = 512  # feature dim
C = 100000  # classes
S = 64.0
MARGIN = 0.5
COS_M = math.cos(MARGIN)
SIN_M = math.sin(MARGIN)
TH = math.cos(math.pi - MARGIN)
MM = math.sin(math.pi - MARGIN) * MARGIN

NCORES = 8
CH = 512  # classes per chunk (one PSUM bank of fp32)
NCH = 25  # chunks per core
CSH = CH * NCH  # 12800 padded classes per core
CPAD = CSH * NCORES  # 102400
KB = D // 128  # 4 contraction blocks
NB = B // 128  # 4 batch blocks
SHIFT = 64.0  # fixed logsumexp shift (logits = S*cos <= 64)

_CACHE = {}


def _fix_act_tables():
    """Make both Exp and Ln resolve to the one table set containing both.

    bass picks the first act-function set containing a needed function; by
    default Exp -> 'exp_and_others' and Ln -> 'natural_log' which thrashes the
    ACT table RAMs (~1.3us per reload, dozens of reloads).  Blank those two
    sets in the cached map (same dict object is returned every call) so both
    functions resolve to 'natural_log_exp_and_others'.  Set *indices* are
    untouched, so the act_func_set_id stays consistent with act_info.json.
    """
    import concourse.hw_specs as hw_specs

    tables = hw_specs.get_activation_tables("gen3")
    for name in ("exp_and_others", "natural_log"):
        if name in tables and "natural_log_exp_and_others" in tables:
            tables[name].clear()


def _build_nc(repeat=1, mm_order="ci_inner", host_norm=False, sup_n=3, pm_bufs=2,
              super_dma=False, q_fold=False, exp_inplace=False, split_first=False):
    import concourse.bass as bass
    import concourse.tile as tile
    from concourse import bacc, mybir

    _fix_act_tables()
    nc = bacc.Bacc(
        "TRN2",
        target_bir_lowering=False,
        debug=False,
        enable_asserts=False,
        num_devices=NCORES,
    )
    bf16 = mybir.dt.bfloat16
    f32 = mybir.dt.float32

    # xnt[d, b] = normalized-x transposed; wt[p, j, k, c] = wn-shard laid out so
    # each 512-class chunk is one contiguous 4KB run per partition.
    xnt = nc.dram_tensor("xnt", [D, B], bf16, kind="ExternalInput").ap()
    wt = nc.dram_tensor("wt", [128, NCH, KB, CH], bf16, kind="ExternalInput").ap()
    s_out = nc.dram_tensor("s_out", [NB, 128], f32, kind="ExternalOutput").ap()

    from contextlib import ExitStack, nullcontext

    with tile.TileContext(nc) as tc, ExitStack() as ctx:
        singles = ctx.enter_context(tc.tile_pool(name="singles", bufs=1))
        wpool = ctx.enter_context(tc.tile_pool(name="wpool", bufs=5))
        qpool = ctx.enter_context(tc.tile_pool(name="qpool", bufs=3))
        rwpool = ctx.enter_context(tc.tile_pool(name="rwpool", bufs=3))
        tpool = ctx.enter_context(tc.tile_pool(name="tpool", bufs=3))
        escrp = ctx.enter_context(tc.tile_pool(name="escr", bufs=4))
        wnpool = ctx.enter_context(tc.tile_pool(name="wnpool", bufs=1))
        psn = ctx.enter_context(tc.tile_pool(name="psn", bufs=2, space="PSUM"))
        psm = ctx.enter_context(tc.tile_pool(name="psm", bufs=pm_bufs, space="PSUM"))

        hint = (
            mybir.EngineType.PE,
            mybir.EngineType.Activation,
            mybir.EngineType.DVE,
            mybir.EngineType.Pool,
            mybir.EngineType.SP,
        )
        ctx.enter_context(
            tc.For_i(0, repeat, 1, hint_engines=hint) if repeat > 1 else nullcontext()
        )

        # x (stationary operand of every matmul): [p, k, b], b-contiguous.
        xs = singles.tile([128, KB, B], bf16)
        nc.sync.dma_start(out=xs[:], in_=xnt.rearrange("(k p) b -> p k b", p=128))

        ones_t = singles.tile([128, 128], bf16)
        nc.vector.memset(ones_t[:], 1.0)

        # per-partition bias vectors (activation bias must be an AP)
        eps_b = singles.tile([128, 1], f32)
        nc.vector.memset(eps_b[:], 1e-12)
        zero_b = singles.tile([128, 1], f32)
        nc.vector.memset(zero_b[:], 0.0)
        nshift_b = singles.tile([128, 1], f32)
        nc.vector.memset(nshift_b[:], -SHIFT)

        # supers: groups of chunks sharing one multi-bank PSUM tile + one exp
        supers = []
        c0 = 0
        while c0 < NCH:
            n = min(sup_n, NCH - c0)
            supers.append(list(range(c0, c0 + n)))
            c0 += n
        NSUP = len(supers)

        # accum_out landing area: one f32 scalar per (batch row, super).
        s_parts = singles.tile([128, NB * NSUP], f32)

        # Per-chunk resident normalized weights (distinct tiles -> no false WAR).
        wns = [
            wnpool.tile([128, KB, CH], bf16, tag=f"wn{j}", name=f"wn{j}")
            for j in range(NCH)
        ]

        for si, sup in enumerate(supers):
            ns = len(sup)
            # ---- produce normalized weights for this super's chunks ----
            wsup = None
            if super_dma and not host_norm:
                wsup = wpool.tile([128, len(sup), KB, CH], bf16, tag="wt",
                                  name=f"wsup{si}", padded_shape=[128, sup_n, KB, CH])
                nc.sync.dma_start(out=wsup[:], in_=wt[:, sup[0] : sup[0] + ns, :, :])
            for ji, j in enumerate(sup):
                if host_norm:
                    # diagnostic variant: wt arrives pre-normalized from host
                    nc.sync.dma_start(out=wns[j][:], in_=wt[:, j, :, :])
                    continue
                if wsup is not None:
                    wtile = wsup[:, ji, :, :]
                else:
                    wtile = wpool.tile([128, KB, CH], bf16, tag="wt", name=f"wt{j}")
                    if split_first and j == 0:
                        for k in range(KB):
                            nc.sync.dma_start(
                                out=wtile[:, k, :], in_=wt[:, j, k, :]
                            )
                    else:
                        nc.sync.dma_start(out=wtile[:], in_=wt[:, j, :, :])

                # q = w*w (bf16: norm err ~0.4%/sqrt(512))
                q = qpool.tile([128, KB, CH], bf16, tag="q", name=f"q{j}")
                nc.vector.tensor_mul(q[:], wtile[:], wtile[:])

                if q_fold:
                    # fold the 4 k-planes on DVE -> single norm matmul (K=128)
                    qa = qpool.tile([128, 2, CH], bf16, tag="qa", name=f"qa{j}")
                    nc.vector.tensor_add(qa[:, 0, :], q[:, 0, :], q[:, 1, :])
                    nc.vector.tensor_add(qa[:, 1, :], q[:, 2, :], q[:, 3, :])
                    qf = qpool.tile([128, CH], bf16, tag="qf", name=f"qf{j}")
                    nc.vector.tensor_add(qf[:], qa[:, 0, :], qa[:, 1, :])
                    pn = psn.tile([128, CH], f32, tag="pn", name=f"pn{j}")
                    nc.tensor.matmul(pn[:], lhsT=ones_t[:], rhs=qf[:], start=True, stop=True)
                else:
                    # n2[c] broadcast to all 128 partitions via ones.T @ q
                    pn = psn.tile([128, CH], f32, tag="pn", name=f"pn{j}")
                    for k in range(KB):
                        nc.tensor.matmul(
                            pn[:],
                            lhsT=ones_t[:],
                            rhs=q[:, k, :],
                            start=(k == 0),
                            stop=(k == KB - 1),
                        )

                # rw = (n2+eps)^-0.5 = exp(-0.5*ln(n2+eps)); Ln+Exp share one
                # ACT table set (see _fix_act_tables) -> no table thrash.
                lnt = tpool.tile([128, CH], f32, tag="lnt", name=f"lnt{j}")
                nc.scalar.activation(
                    lnt[:],
                    pn[:],
                    mybir.ActivationFunctionType.Ln,
                    bias=eps_b[:],
                    scale=1.0,
                )
                rw = rwpool.tile([128, CH], bf16, tag="rw", name=f"rw{j}")
                nc.scalar.activation(
                    rw[:],
                    lnt[:],
                    mybir.ActivationFunctionType.Exp,
                    bias=zero_b[:],
                    scale=-0.5,
                )

                # wn = w * rw (one DVE op; rw broadcast over k via step-0 AP)
                rw_b = bass.AP(
                    tensor=rw.tensor,
                    offset=rw.offset,
                    ap=[rw.ap[0], [0, KB], rw.ap[1]],
                )
                nc.vector.tensor_mul(wns[j][:], wtile[:], rw_b)

            # ---- logits + exp for this super across all batch blocks ----
            for nb in range(NB):
                pm = psm.tile([128, sup_n * CH], f32, tag="pm", name=f"pm{si}_{nb}")
                # k outer: the stationary operand repeats across the chunks of
                # the super, maximizing LDWEIGHTS reuse/overlap
                if mm_order == "k_outer":
                    for k in range(KB):
                        for ci, j in enumerate(sup):
                            nc.tensor.matmul(
                                pm[:, ci * CH : (ci + 1) * CH],
                                lhsT=xs[:, k, nb * 128 : (nb + 1) * 128],
                                rhs=wns[j][:, k, :],
                                start=(k == 0),
                                stop=(k == KB - 1),
                            )
                else:
                    for ci, j in enumerate(sup):
                        for k in range(KB):
                            nc.tensor.matmul(
                                pm[:, ci * CH : (ci + 1) * CH],
                                lhsT=xs[:, k, nb * 128 : (nb + 1) * 128],
                                rhs=wns[j][:, k, :],
                                start=(k == 0),
                                stop=(k == KB - 1),
                            )
                if exp_inplace:
                    nc.scalar.activation(
                        pm[:, : ns * CH],
                        pm[:, : ns * CH],
                        mybir.ActivationFunctionType.Exp,
                        bias=nshift_b[:],
                        scale=S,
                        accum_out=s_parts[:, nb * NSUP + si : nb * NSUP + si + 1],
                    )
                else:
                    es = escrp.tile([128, sup_n * CH], bf16, tag="es", name=f"es{si}_{nb}")
                    nc.scalar.activation(
                        es[:, : ns * CH],
                        pm[:, : ns * CH],
                        mybir.ActivationFunctionType.Exp,
                        bias=nshift_b[:],
                        scale=S,
                        accum_out=s_parts[:, nb * NSUP + si : nb * NSUP + si + 1],
                    )

        s_fin = singles.tile([128, NB], f32)
        nc.vector.tensor_reduce(
            out=s_fin[:],
            in_=s_parts[:].rearrange("p (nb nsup) -> p nb nsup", nb=NB),
            axis=mybir.AxisListType.X,
            op=mybir.AluOpType.add,
        )
        nc.sync.dma_start(out=s_out.rearrange("nb p -> p nb"), in_=s_fin[:])

    nc.compile()
    return nc


def _get_nc():
    if "nc" not in _CACHE:
        _CACHE["nc"] = _build_nc()
    return _CACHE["nc"]


def _prep_inputs(x, weights):
    """Host-side shard/layout prep: normalize x, shard+transpose+cast W."""
    x = np.asarray(x, dtype=np.float32)
    w = np.asarray(weights, dtype=np.float32)

    xn = x / np.linalg.norm(x.astype(np.float64), axis=1, keepdims=True)
    xnt = np.ascontiguousarray(xn.T).astype(ml_dtypes.bfloat16)

    wpad = np.zeros((CPAD, D), dtype=np.float32)
    wpad[:C] = w
    wt_maps = []
    for i in range(NCORES):
        shard = wpad[i * CSH : (i + 1) * CSH]  # [12800, 512]
        # -> [p, j, k, c] with [j,k,c] contiguous per partition
        arr = shard.reshape(NCH, CH, KB, 128).transpose(3, 0, 2, 1)
        wt_maps.append(np.ascontiguousarray(arr).astype(ml_dtypes.bfloat16))
    return xnt, wt_maps


def _run_on_device(xnt, wt_maps, trace=False):
    from concourse.bass_utils import run_bass_kernel_spmd

    nc = _get_nc()
    in_maps = [{"xnt": xnt, "wt": wt_maps[i]} for i in range(NCORES)]
    res = run_bass_kernel_spmd(
        nc, in_maps, core_ids=list(range(NCORES)), trace=trace
    )
    _CACHE["last_results"] = res
    return [r["s_out"].reshape(B).astype(np.float64) for r in res.results]


def kernel(x, weights, targets, _trace=False):
    x = np.asarray(x)
    weights = np.asarray(weights)
    targets = np.asarray(targets).astype(np.int64)

    xnt, wt_maps = _prep_inputs(x, weights)
    s_shards = _run_on_device(xnt, wt_maps, trace=_trace)

    # ---- host combine (f64, ~0.5 MFLOP total) ----
    s_total = np.sum(s_shards, axis=0)  # [B]
    # remove zero-pad classes: each contributes exp(0*S - SHIFT) exactly
    npad = CPAD - C
    s_total = s_total - npad * math.exp(-SHIFT)

    xf = x.astype(np.float64)
    xn = xf / np.linalg.norm(xf, axis=1, keepdims=True)
    wtg = weights.astype(np.float64)[targets]  # [B, D] gathered target rows
    wtg = wtg / np.linalg.norm(wtg, axis=1, keepdims=True)
    cos_t = np.einsum("bd,bd->b", xn, wtg)

    sin_t = np.sqrt(np.clip(1.0 - cos_t * cos_t, 0.0, 1.0))
    phi = cos_t * COS_M - sin_t * SIN_M
    psi = np.where(cos_t > TH, phi, cos_t - MM)

    # swap the target term: remove exp(S*cos_t), add exp(S*psi)
    s_adj = s_total - np.exp(S * cos_t - SHIFT) + np.exp(S * psi - SHIFT)
    lse = SHIFT + np.log(s_adj)
    loss = np.mean(lse - S * psi)
    return np.float32(loss)



# revision 2
# speedup vs baseline: 1.0353x; 1.0353x over previous
"""ArcFace loss kernel for Trainium2, class-sharded across 8 NeuronCores.

Strategy (vocab/tensor parallel per the module's own sharding):
  - Shard the class axis of `weights` (100000 classes -> 8 x 12800, zero-padded).
  - Host sends x-hat (normalized x) and raw w, both scaled by 8 and cast to
    fp8e4m3 (the x8 keeps typical N(0, 1/sqrt(D)) values out of fp8 subnormals).
  - Each core: normalize its weight shard on device (squares -> ones-matmul
    partition-reduce -> rsqrt via exp(-0.5*ln + ln 8)), wn = w * rw in fp8,
    then DoubleRow fp8 matmul dot[b, c] = (8 x-hat).(8 w-hat) = 64*cos, and
    ScalarE exp(dot - 64) with accum_out producing per-row partial exp-sums.
  - Host: sum the 8 partial exp-sums (f64), fix up the 512 target-class
    entries with the ArcFace margin (cos(theta+m) correction), and take the
    mean cross-entropy.  A fixed shift of -64 (= -S, since cos <= 1) replaces
    the usual running max, so no cross-core max reduction is needed.

Norm ACT work is batched (Ln over 2-chunk PSUM groups, Exp over 4-chunk SBUF
groups) and Ln+Exp share one ACT table set (see _fix_act_tables), so the
ScalarE runs with no table reloads.
"""

import math

import ml_dtypes
import numpy as np

# Problem constants (hardcoded per contract; kernel.py must be self-contained).
B = 512  # batch
D = 512  # feature dim
C = 100000  # classes
S = 64.0
MARGIN = 0.5
COS_M = math.cos(MARGIN)
SIN_M = math.sin(MARGIN)
TH = math.cos(math.pi - MARGIN)
MM = math.sin(math.pi - MARGIN) * MARGIN

NCORES = 8
CH = 512  # classes per chunk (one PSUM bank of fp32)
NCH = 25  # chunks per core
CSH = CH * NCH  # 12800 padded classes per core
CPAD = CSH * NCORES  # 102400
KB = D // 128  # 4 contraction blocks
NB = B // 128  # 4 batch blocks
SHIFT = 64.0  # fixed logsumexp shift (logits = S*cos <= 64)
SUP = 3  # chunks per main-matmul super (PSUM banks per pm tile)
NSUP = (NCH + SUP - 1) // SUP
LNG = 2  # chunks per Ln group (psn PSUM banks)
EXG = 4  # chunks per norm-Exp group (must be multiple of LNG)
F8SCALE = 8.0  # host-side power-of-2 scale keeping fp8 values normal-range

_CACHE = {}


def _fix_act_tables():
    """Make both Exp and Ln resolve to the one table set containing both.

    bass picks the first act-function set containing a needed function; by
    default Exp -> 'exp_and_others' and Ln -> 'natural_log' which thrashes the
    ACT table RAMs (~2.7us per reload, dozens of reloads).  Blank those two
    sets in the cached map (same dict object is returned every call) so both
    functions resolve to 'natural_log_exp_and_others'.  Set *indices* are
    untouched, so the act_func_set_id stays consistent with act_info.json.
    """
    import concourse.hw_specs as hw_specs

    tables = hw_specs.get_activation_tables("gen3")
    for name in ("exp_and_others", "natural_log"):
        if name in tables and "natural_log_exp_and_others" in tables:
            tables[name].clear()


def _build_nc(repeat=1):
    import concourse.bass as bass
    import concourse.tile as tile
    from concourse import bacc, mybir

    _fix_act_tables()
    nc = bacc.Bacc(
        "TRN2",
        target_bir_lowering=False,
        debug=False,
        enable_asserts=False,
        num_devices=NCORES,
    )
    f8 = mybir.dt.float8e4
    bf16 = mybir.dt.bfloat16
    f32 = mybir.dt.float32
    DR = mybir.MatmulPerfMode.DoubleRow

    # xnt[d, b] = 8 * normalized-x transposed; wt[p, j, k, c] = 8 * w-shard
    # laid out so each 512-class chunk is one contiguous 2KB run per partition.
    xnt = nc.dram_tensor("xnt", [D, B], f8, kind="ExternalInput").ap()
    wt = nc.dram_tensor("wt", [128, NCH, KB, CH], f8, kind="ExternalInput").ap()
    s_out = nc.dram_tensor("s_out", [NB, 128], f32, kind="ExternalOutput").ap()

    from contextlib import ExitStack, nullcontext

    with tile.TileContext(nc) as tc, ExitStack() as ctx:
        singles = ctx.enter_context(tc.tile_pool(name="singles", bufs=1))
        wpool = ctx.enter_context(tc.tile_pool(name="wpool", bufs=6))
        qpool = ctx.enter_context(tc.tile_pool(name="qpool", bufs=3))
        escrp = ctx.enter_context(tc.tile_pool(name="escr", bufs=4))
        wnpool = ctx.enter_context(tc.tile_pool(name="wnpool", bufs=1))
        psn = ctx.enter_context(tc.tile_pool(name="psn", bufs=1, space="PSUM"))
        psm = ctx.enter_context(tc.tile_pool(name="psm", bufs=2, space="PSUM"))

        hint = (
            mybir.EngineType.PE,
            mybir.EngineType.Activation,
            mybir.EngineType.DVE,
            mybir.EngineType.Pool,
            mybir.EngineType.SP,
        )
        ctx.enter_context(
            tc.For_i(0, repeat, 1, hint_engines=hint) if repeat > 1 else nullcontext()
        )

        # x (stationary operand of every main matmul): [p, k, b], b-contiguous.
        xs = singles.tile([128, KB, B], f8)
        nc.sync.dma_start(out=xs[:], in_=xnt.rearrange("(k p) b -> p k b", p=128))

        ones_t = singles.tile([128, 128], bf16)
        nc.vector.memset(ones_t[:], 1.0)

        # per-partition bias vectors (activation bias must be an AP)
        eps_b = singles.tile([128, 1], f32)
        nc.vector.memset(eps_b[:], 1e-12)
        l8_b = singles.tile([128, 1], f32)
        nc.vector.memset(l8_b[:], math.log(F8SCALE))
        nshift_b = singles.tile([128, 1], f32)
        nc.vector.memset(nshift_b[:], -SHIFT)

        # accum_out landing area: one f32 scalar per (batch row, super).
        s_parts = singles.tile([128, NB * NSUP], f32)

        # norm staging: ln(n2) per class, rw = 8/sqrt(n2') per class
        lnt = singles.tile([128, NCH, CH], f32)
        rw_all = singles.tile([128, NCH, CH], bf16)

        # Per-chunk resident normalized fp8 weights.
        wns = [
            wnpool.tile([128, KB, CH], f8, tag=f"wn{j}", name=f"wn{j}")
            for j in range(NCH)
        ]
        wts = {}

        ln_groups = [(g * LNG, min(g * LNG + LNG, NCH)) for g in range((NCH + LNG - 1) // LNG)]
        exp_groups = [(e * EXG, min(e * EXG + EXG, NCH)) for e in range((NCH + EXG - 1) // EXG)]

        def emit_ln_group(g):
            j0, j1 = ln_groups[g]
            n = j1 - j0
            pn = psn.tile([128, LNG, CH], f32, tag="pn", name=f"pn{g}")
            for ci, j in enumerate(range(j0, j1)):
                wtile = wpool.tile([128, KB, CH], f8, tag="wt", name=f"wt{j}")
                nc.sync.dma_start(out=wtile[:], in_=wt[:, j, :, :])
                wts[j] = wtile
                # q = w*w (fp8 in, bf16 out)
                q = qpool.tile([128, KB, CH], bf16, tag="q", name=f"q{j}")
                nc.vector.tensor_mul(q[:], wtile[:], wtile[:])
                # n2' broadcast to all 128 partitions via ones.T @ q
                for k in range(KB):
                    nc.tensor.matmul(
                        pn[:, ci, :],
                        lhsT=ones_t[:],
                        rhs=q[:, k, :],
                        start=(k == 0),
                        stop=(k == KB - 1),
                    )
            nc.scalar.activation(
                lnt[:, j0:j1, :],
                pn[:, :n, :],
                mybir.ActivationFunctionType.Ln,
                bias=eps_b[:],
                scale=1.0,
            )

        def emit_exp_group(e):
            j0, j1 = exp_groups[e]
            # rw = 8 * rsqrt(n2') = exp(-0.5*ln(n2') + ln 8)
            nc.scalar.activation(
                rw_all[:, j0:j1, :],
                lnt[:, j0:j1, :],
                mybir.ActivationFunctionType.Exp,
                bias=l8_b[:],
                scale=-0.5,
            )
            for j in range(j0, j1):
                rwj = rw_all[:, j, :]
                rw_b = bass.AP(
                    tensor=rw_all.tensor,
                    offset=rwj.offset,
                    ap=[rwj.ap[0], [0, KB], rwj.ap[-1]],
                )
                # wn = w * rw (fp8 x bf16-broadcast -> fp8)
                nc.vector.tensor_mul(wns[j][:], wts[j][:], rw_b)

        supers = [list(range(c0, min(c0 + SUP, NCH))) for c0 in range(0, NCH, SUP)]

        emitted_ln = 0
        emitted_exp = 0
        for si, sup in enumerate(supers):
            # advance the norm pipeline until every chunk of this super has wn
            while emitted_exp * EXG < sup[-1] + 1:
                e = emitted_exp
                for g in range(e * EXG // LNG, min((e + 1) * EXG // LNG, len(ln_groups))):
                    if g >= emitted_ln:
                        emit_ln_group(g)
                        emitted_ln = g + 1
                emit_exp_group(e)
                emitted_exp += 1

            # logits + exp for this super across all batch blocks
            ns = len(sup)
            for nb in range(NB):
                pm = psm.tile([128, SUP * CH], f32, tag="pm", name=f"pm{si}_{nb}")
                for ci, j in enumerate(sup):
                    for t in range(KB // 2):
                        nc.tensor.matmul(
                            pm[:, ci * CH : (ci + 1) * CH],
                            lhsT=xs[:, 2 * t : 2 * t + 2, nb * 128 : (nb + 1) * 128],
                            rhs=wns[j][:, 2 * t : 2 * t + 2, :],
                            start=(t == 0),
                            stop=(t == KB // 2 - 1),
                            perf_mode=DR,
                        )
                es = escrp.tile([128, SUP * CH], bf16, tag="es", name=f"es{si}_{nb}")
                nc.scalar.activation(
                    es[:, : ns * CH],
                    pm[:, : ns * CH],
                    mybir.ActivationFunctionType.Exp,
                    bias=nshift_b[:],
                    scale=1.0,
                    accum_out=s_parts[:, nb * NSUP + si : nb * NSUP + si + 1],
                )

        s_fin = singles.tile([128, NB], f32)
        nc.vector.tensor_reduce(
            out=s_fin[:],
            in_=s_parts[:].rearrange("p (nb nsup) -> p nb nsup", nb=NB),
            axis=mybir.AxisListType.X,
            op=mybir.AluOpType.add,
        )
        nc.sync.dma_start(out=s_out.rearrange("nb p -> p nb"), in_=s_fin[:])

    nc.compile()
    return nc


def _get_nc():
    if "nc" not in _CACHE:
        _CACHE["nc"] = _build_nc()
    return _CACHE["nc"]


def _prep_inputs(x, weights):
    """Host-side shard/layout prep: normalize x, shard+transpose+cast W to fp8."""
    x = np.asarray(x, dtype=np.float32)
    w = np.asarray(weights, dtype=np.float32)

    xn = x / np.linalg.norm(x.astype(np.float64), axis=1, keepdims=True)
    xnt = np.ascontiguousarray(xn.T * F8SCALE).astype(ml_dtypes.float8_e4m3)

    wpad = np.zeros((CPAD, D), dtype=np.float32)
    wpad[:C] = w * F8SCALE
    wt_maps = []
    for i in range(NCORES):
        shard = wpad[i * CSH : (i + 1) * CSH]  # [12800, 512]
        # -> [p, j, k, c] with [j,k,c] contiguous per partition
        arr = shard.reshape(NCH, CH, KB, 128).transpose(3, 0, 2, 1)
        wt_maps.append(np.ascontiguousarray(arr).astype(ml_dtypes.float8_e4m3))
    return xnt, wt_maps


def _run_on_device(xnt, wt_maps, trace=False):
    from concourse.bass_utils import run_bass_kernel_spmd

    nc = _get_nc()
    in_maps = [{"xnt": xnt, "wt": wt_maps[i]} for i in range(NCORES)]
    res = run_bass_kernel_spmd(
        nc, in_maps, core_ids=list(range(NCORES)), trace=trace
    )
    _CACHE["last_results"] = res
    return [r["s_out"].reshape(B).astype(np.float64) for r in res.results]


def kernel(x, weights, targets, _trace=False):
    x = np.asarray(x)
    weights = np.asarray(weights)
    targets = np.asarray(targets).astype(np.int64)

    xnt, wt_maps = _prep_inputs(x, weights)
    s_shards = _run_on_device(xnt, wt_maps, trace=_trace)

    # ---- host combine (f64, ~0.5 MFLOP total) ----
    s_total = np.sum(s_shards, axis=0)  # [B]
    # remove zero-pad classes: each contributes exp(0*S - SHIFT) exactly
    npad = CPAD - C
    s_total = s_total - npad * math.exp(-SHIFT)

    xf = x.astype(np.float64)
    xn = xf / np.linalg.norm(xf, axis=1, keepdims=True)
    wtg = weights.astype(np.float64)[targets]  # [B, D] gathered target rows
    wtg = wtg / np.linalg.norm(wtg, axis=1, keepdims=True)
    cos_t = np.einsum("bd,bd->b", xn, wtg)

    sin_t = np.sqrt(np.clip(1.0 - cos_t * cos_t, 0.0, 1.0))
    phi = cos_t * COS_M - sin_t * SIN_M
    psi = np.where(cos_t > TH, phi, cos_t - MM)

    # swap the target term: remove exp(S*cos_t), add exp(S*psi)
    s_adj = s_total - np.exp(S * cos_t - SHIFT) + np.exp(S * psi - SHIFT)
    lse = SHIFT + np.log(s_adj)
    loss = np.mean(lse - S * psi)
    return np.float32(loss)
